# revision 7
# baseline (speedup 1.0000x reference)
"""Bass/Trainium2 kernel for nn_CD_49555332661898 (RKD-style per-class HuberDist).

Math (per class c, with mask m = targets[:, c] in {0,1}):
  ds = sqrt(max(d2(f_s), EPS)) with zero diag; masked pair sums; dsn = ds/mean;
  loss_c = sum(huber(dsn - dtn) * M) / n^2, summed over classes with n > 1.

Key identity used on device: with masked features g = m*f laid out [L, N]
(transposed) and two augmentation rows (m, -m*sq/2), a single PE accumulation
computes  P[i,j] = -(m_i m_j d2_ij)/2  directly.  Then
  ds~ = sqrt(-2*P + EPS) = m_i m_j ds_ij  (+sqrt(EPS) leakage on masked pairs,
negligible), and all downstream elementwise work needs no masks at all.

Sharding: classes are split 10-per-core across 8 NeuronCores (embarrassingly
parallel); each core returns per-class partial accumulators ([128, 8] per
class) and the host does the final tiny reduction.
"""

import os
import sys

import numpy as np

for _p in ("/opt/trn_rl_repo", "/root/.axon_site/_ro/trn_rl_repo"):
    if os.path.isdir(_p) and _p not in sys.path:
        sys.path.insert(0, _p)

import concourse.bacc as bacc
import concourse.tile as tile
import concourse.mybir as mybir
from concourse import bass_isa
from concourse.bass_utils import run_bass_kernel_spmd

F32 = mybir.dt.float32
F32R = mybir.dt.float32r
F16 = mybir.dt.float16
AOP = mybir.AluOpType
AFT = mybir.ActivationFunctionType
AX = mybir.AxisListType

N, C, L = 512, 80, 512
NCORES = 8
CPC = C // NCORES          # classes per core
NB = 4                     # 128-row blocks per [512, 512] matrix
EPS = 1e-8
SLACK = 0.25               # keeps the (rounding-noisy) diagonal of d2 positive


def _round_fp32r(a: np.ndarray) -> np.ndarray:
    """Round fp32 to the fp32r grid (11-bit mantissa, RTNE, low 12 bits 0)."""
    u = np.ascontiguousarray(a, dtype=np.float32).view(np.uint32)
    bias = np.uint32(0x7FF) + ((u >> np.uint32(12)) & np.uint32(1))
    return ((u + bias) & np.uint32(0xFFFFF000)).view(np.float32)


def _build_program():
    nc = bacc.Bacc("TRN2", target_bir_lowering=False, debug=False,
                   num_devices=NCORES)

    fs_in = nc.dram_tensor("fs", [CPC, 128, NB, 512], F32R,
                           kind="ExternalInput").ap()
    ft_in = nc.dram_tensor("ft", [CPC, 128, NB, 512], F32R,
                           kind="ExternalInput").ap()
    augl_s = nc.dram_tensor("augl_s", [2, CPC, 512], F32R,
                            kind="ExternalInput").ap()
    augr_s = nc.dram_tensor("augr_s", [2, CPC, 512], F32R,
                            kind="ExternalInput").ap()
    augl_t = nc.dram_tensor("augl_t", [2, CPC, 512], F32R,
                            kind="ExternalInput").ap()
    augr_t = nc.dram_tensor("augr_t", [2, CPC, 512], F32R,
                            kind="ExternalInput").ap()
    consts = nc.dram_tensor("consts", [128, 16], F32,
                            kind="ExternalInput").ap()
    acc_out = nc.dram_tensor("acc_out", [CPC, 128, 8], F32,
                             kind="ExternalOutput").ap()
    debug = bool(int(os.environ.get("CD_DEBUG", "0")))
    if debug:
        dbg_rs = nc.dram_tensor("dbg_rs", [128, 8], F32,
                                kind="ExternalOutput").ap()
        dbg_rsum = nc.dram_tensor("dbg_rsum", [128, 8], F32,
                                  kind="ExternalOutput").ap()
        dbg_stats = nc.dram_tensor("dbg_stats", [128, 6], F32,
                                   kind="ExternalOutput").ap()
        dbg_ds = nc.dram_tensor("dbg_ds", [128, 512], F16,
                                kind="ExternalOutput").ap()

    with tile.TileContext(nc) as tc:
        with tc.tile_pool(name="feat", bufs=3) as featp, \
             tc.tile_pool(name="dmat", bufs=2) as dmatp, \
             tc.tile_pool(name="smol", bufs=2) as smolp, \
             tc.tile_pool(name="stat", bufs=1) as statp, \
             tc.tile_pool(name="psg", bufs=8, space="PSUM") as psg:

            CONSTS = statp.tile([128, 16], F32)
            nc.sync.dma_start(out=CONSTS[:], in_=consts)
            EPST = statp.tile([128, 1], F32)
            nc.vector.memset(EPST[:], EPS)

            for c in range(CPC):
                FS = featp.tile([128, NB, 512], F32R, tag="FS")
                nc.sync.dma_start(out=FS[:], in_=fs_in[c])
                FT = featp.tile([128, NB, 512], F32R, tag="FT")
                nc.sync.dma_start(out=FT[:], in_=ft_in[c])
                ALS = smolp.tile([2, 512], F32R, tag="ALS")
                nc.sync.dma_start(out=ALS[:], in_=augl_s[:, c, :])
                ARS = smolp.tile([2, 512], F32R, tag="ARS")
                nc.sync.dma_start(out=ARS[:], in_=augr_s[:, c, :])
                ALT = smolp.tile([2, 512], F32R, tag="ALT")
                nc.sync.dma_start(out=ALT[:], in_=augl_t[:, c, :])
                ART = smolp.tile([2, 512], F32R, tag="ART")
                nc.sync.dma_start(out=ART[:], in_=augr_t[:, c, :])

                DS = dmatp.tile([128, NB, 512], F16, tag="DS")
                DT = dmatp.tile([128, NB, 512], F16, tag="DT")
                RS = smolp.tile([128, 8], F32, tag="RS")

                for (F, AL, AR, D, rsoff) in (
                    (FS, ALS, ARS, DS, 0),
                    (FT, ALT, ART, DT, 4),
                ):
                    for b in range(NB):
                        PS = psg.tile([128, 512], F32, tag="PS")
                        js = slice(128 * b, 128 * b + 128)
                        for kk in range(NB):
                            nc.tensor.matmul(PS[:], F[:, kk, js], F[:, kk, :],
                                             start=(kk == 0), stop=False)
                        nc.tensor.matmul(PS[:], AL[:, js], AR[:, :],
                                         start=False, stop=True)
                        # ds~ = sqrt(m_i m_j d2 + EPS); rowsums accumulate free
                        nc.scalar.activation(D[:, b, :], PS[:], AFT.Sqrt,
                                             bias=EPST[:], scale=-2.0,
                                             accum_out=RS[:, rsoff + b:rsoff + b + 1])

                # total sums -> means -> alphas (per class scalars, bcast on
                # all 128 partitions)
                RSUM = smolp.tile([128, 8], F32, tag="RSUM")
                nc.gpsimd.partition_all_reduce(RSUM[:], RS[:], 128,
                                               bass_isa.ReduceOp.add)
                SUMS = smolp.tile([128, 2], F32, tag="SUMS")
                nc.vector.tensor_reduce(SUMS[:],
                                        RSUM[:].rearrange("p (s b) -> p s b", s=2),
                                        AX.X, AOP.add)
                MEANS = smolp.tile([128, 2], F32, tag="MEANS")
                nc.vector.tensor_scalar(MEANS[:], SUMS[:],
                                        CONSTS[:, c:c + 1], EPS,
                                        AOP.mult, AOP.max)
                ALPH = smolp.tile([128, 2], F32, tag="ALPH")
                nc.vector.reciprocal(ALPH[:], MEANS[:])

                if debug and c == 0:
                    nc.sync.dma_start(out=dbg_rs, in_=RS[:])
                    nc.sync.dma_start(out=dbg_rsum, in_=RSUM[:])
                    STATS = smolp.tile([128, 6], F32, tag="STATS")
                    nc.vector.tensor_copy(STATS[:, 0:2], SUMS[:])
                    nc.vector.tensor_copy(STATS[:, 2:4], MEANS[:])
                    nc.vector.tensor_copy(STATS[:, 4:6], ALPH[:])
                    nc.sync.dma_start(out=dbg_stats, in_=STATS[:])
                    nc.sync.dma_start(out=dbg_ds, in_=DS[:, 0, :])

                ACC = smolp.tile([128, 8], F32, tag="ACC")
                for b in range(NB):
                    nc.vector.tensor_scalar(DS[:, b, :], DS[:, b, :],
                                            ALPH[:, 0:1], None, AOP.mult)
                    nc.vector.tensor_scalar(DT[:, b, :], DT[:, b, :],
                                            ALPH[:, 1:2], None, AOP.mult)
                    nc.vector.tensor_tensor(DT[:, b, :], DS[:, b, :],
                                            DT[:, b, :], AOP.subtract)
                    SCR = smolp.tile([128, 512], F16, tag="SCR")
                    nc.scalar.activation(SCR[:], DT[:, b, :], AFT.Square,
                                         accum_out=ACC[:, b:b + 1])
                    nc.vector.tensor_reduce(ACC[:, 4 + b:5 + b], DT[:, b, :],
                                            AX.X, AOP.max,
                                            apply_absolute_value=True)
                nc.sync.dma_start(out=acc_out[c], in_=ACC[:])

    nc.compile()
    return nc


_CACHED_NC = None


def kernel(le_student: np.ndarray, le_teacher: np.ndarray,
           targets: np.ndarray) -> np.ndarray:
    global _CACHED_NC

    m = targets.T.astype(np.float32)                      # [C, N]
    nvec = m.sum(axis=1)                                  # positives per class

    def prep(le):
        # [N, C, L] -> masked, transposed, fp32r-rounded [C, L, N]
        g = np.ascontiguousarray(le.transpose(1, 2, 0)).astype(np.float32)
        g *= m[:, None, :]
        g = _round_fp32r(g)
        sq = (g.astype(np.float64) ** 2).sum(axis=1)      # [C, N]
        # Snap sq+SLACK to the f32r grid FIRST; then *0.5 and *m are exact,
        # so the device-side aug rows carry no extra rounding (the diagonal
        # of d2 stays >= SLACK instead of going sqrt(negative) -> NaN).
        sq_r = _round_fp32r((sq + SLACK).astype(np.float32))
        sqrow = -0.5 * sq_r * m                           # [C, N]
        feats = np.ascontiguousarray(
            g.reshape(C, NB, 128, N).transpose(0, 2, 1, 3))  # [C, 128, NB, N]
        return feats, sqrow

    feats_s, sqrow_s = prep(le_student)
    feats_t, sqrow_t = prep(le_teacher)

    inv_cnt = 1.0 / np.maximum(nvec * (nvec - 1.0), 1.0)  # [C]
    wv = np.where(nvec > 1.0, 1.0 / np.maximum(nvec * nvec, 1.0), 0.0)

    in_maps = []
    for k in range(NCORES):
        cs = slice(k * CPC, (k + 1) * CPC)
        consts = np.zeros((128, 16), dtype=np.float32)
        consts[:, 0:CPC] = inv_cnt[cs][None, :]
        in_maps.append({
            "fs": feats_s[cs],
            "ft": feats_t[cs],
            "augl_s": np.ascontiguousarray(
                np.stack([m[cs], sqrow_s[cs]], axis=0)),   # [2, CPC, N]
            "augr_s": np.ascontiguousarray(
                np.stack([sqrow_s[cs], m[cs]], axis=0)),
            "augl_t": np.ascontiguousarray(
                np.stack([m[cs], sqrow_t[cs]], axis=0)),
            "augr_t": np.ascontiguousarray(
                np.stack([sqrow_t[cs], m[cs]], axis=0)),
            "consts": consts,
        })

    if _CACHED_NC is None:
        _CACHED_NC = _build_program()
    nc = _CACHED_NC

    res = run_bass_kernel_spmd(nc, in_maps, core_ids=list(range(NCORES)))
    kernel.last_exec_time_ns = res.exec_time_ns
    kernel.last_in_maps = in_maps

    total = 0.0
    for k in range(NCORES):
        acc = np.asarray(res.results[k]["acc_out"], dtype=np.float64)
        qu = acc[:, :, 0:4].sum(axis=(1, 2))              # [CPC] sum of u^2
        mx = acc[:, :, 4:8].max()
        # huber == 0.5*u^2 exactly when |u| < 1 everywhere (guard below);
        # randn-distributed inputs keep |u| ~ 0.05, ~20 sigma of margin.
        if mx >= 1.0:
            raise AssertionError(
                f"max|dsn-dtn| = {mx} >= 1: huber shortcut invalid")
        w = wv[k * CPC:(k + 1) * CPC]
        total += float((0.5 * qu * w).sum())

    return np.float32(total)


kernel.last_exec_time_ns = None
kernel.last_in_maps = None


if __name__ == "__main__":
    rng = np.random.default_rng(0)
    le_s = rng.standard_normal((N, C, L)).astype(np.float32)
    le_t = rng.standard_normal((N, C, L)).astype(np.float32)
    tg = rng.integers(0, 2, size=(N, C)).astype(np.int32)
    out = kernel(le_student=le_s, le_teacher=le_t, targets=tg)
    print("kernel out:", out, "exec_ns:", kernel.last_exec_time_ns)
